# revision 18
# baseline (speedup 1.0000x reference)
"""Chamfer distance matrix (L2) kernel for 8 Trainium2 NeuronCores.

Problem: xyz1 [B=32, G1=64, N1=32, 3], xyz2 [B=32, G2=64, N2=32, 3] ->
out[b, g1, g2] = mean_n1 min_n2 d + mean_n2 min_n1 d, where
d[n1, n2] = |x - y|^2 between points of group (b, g1) and (b, g2).

Strategy (data-parallel over B, 4 batches per core), software-pipelined
at half-batch granularity (8 g1-tiles): orientation-B min work and the
mean matmuls of half-batch k are emitted after the matmul/staging work
of half-batch k+1, so no engine's in-order stream stalls on a
cross-engine dependency.
  - Host packs points into augmented 5-vectors so one K=5 fp16 matmul
    produces the pairwise squared-distance matrix:
      X' = (|x|^2, 1, -2x),  Y' = (1, |y|^2, y),  d = X'.Y'
    Only orientation A (rows = g1 points) is computed on the PE; the
    rhs layout splits n2 into halves so the first min level folds them.
  - d-tiles are staged to fp16 SBUF by ACT (the only engine that can
    read PSUM while leaving DVE free for min work; Pool cannot access
    PSUM and its TensorTensor is rejected by codegen).
  - Orientation B (min over n1) is obtained WITHOUT matmuls or PSUM:
    one XBAR DMA-transpose per staged [128, 4096] tile-PAIR performs 32
    blocked 128x128 transposes on the DMA engines, assembling d^T in
    fp16 SBUF (i-major layout, whose strides merge back into simple 3D
    access patterns for the fold). All min work runs on DVE at 2x fp16
    with half-batch-wide L1 folds and 8-tile min trees.
  - Means over the 32 points: one wide matmul per orientation per
    half-batch with the block-diagonal (1/32) matrix stationary
    (columns are contracted independently), accumulating orientation
    B's two 16-row halves in PSUM. Raw [4,512]/[8,256] mean grids are
    DMA'd out and the host reassembles Z_A + Z_B^T.
"""

import functools
import numpy as np

import concourse.bass as bass
import concourse.tile as tile
from concourse import bacc, mybir
from concourse import bass_utils

F32 = mybir.dt.float32
F16 = mybir.dt.float16
MIN = mybir.AluOpType.min

B, G, N = 32, 64, 32
NCORES = 8
BPC = B // NCORES          # batches per core
PTS = BPC * G * N          # points per core per set (8192)

CONFIG = {
    "dve_stage": (),       # (pair, slot) halves staged by DVE instead of ACT
}

# Set by test.py to collect an NTFF profile + exec time.
TRACE = False
TRACE_DIR = None
LAST_EXEC_NS = None
LAST_RESULT = None


def _cfg_key(cfg):
    return tuple(sorted((k, tuple(v)) for k, v in cfg.items()))


@functools.lru_cache(maxsize=4)
def _build(cfg_items):
    cfg = dict((k, v) for k, v in cfg_items)
    dve_stage = set(cfg["dve_stage"])

    nc = bacc.Bacc(
        "TRN2", target_bir_lowering=False, debug=False, enable_asserts=False
    )
    xal_d = nc.dram_tensor("xal", [5, PTS], F16, kind="ExternalInput")
    yar_d = nc.dram_tensor("yar", [5, PTS], F16, kind="ExternalInput")
    bo32_d = nc.dram_tensor("bo32", [128, 4], F16, kind="ExternalInput")
    bo16_d = nc.dram_tensor("bo16", [128, 8], F16, kind="ExternalInput")
    za_d = nc.dram_tensor("za", [BPC, 2, 4, 512], F32, kind="ExternalOutput")
    zb_d = nc.dram_tensor("zb", [BPC, 2, 8, 256], F32, kind="ExternalOutput")

    with tile.TileContext(nc) as tc:
        with (
            tc.tile_pool(name="const", bufs=1) as cpool,
            tc.tile_pool(name="io", bufs=2) as iopool,
            tc.tile_pool(name="dpsum", bufs=3, space="PSUM") as dpool,
            tc.tile_pool(name="zpsum", bufs=1, space="PSUM") as zpool,
            tc.tile_pool(name="sa", bufs=3) as sapool,
            tc.tile_pool(name="bt", bufs=2) as btpool,
            tc.tile_pool(name="t1a", bufs=2) as t1apool,
            tc.tile_pool(name="t1b", bufs=2) as t1bpool,
            tc.tile_pool(name="t2", bufs=2) as t2pool,
            tc.tile_pool(name="t3", bufs=2) as t3pool,
            tc.tile_pool(name="t4", bufs=2) as t4pool,
            tc.tile_pool(name="m", bufs=2) as mpool,
            tc.tile_pool(name="zs", bufs=2) as zspool,
        ):
            BO32 = cpool.tile([128, 4], F16)
            nc.gpsimd.dma_start(BO32[:], bo32_d.ap()[:])
            BO16 = cpool.tile([128, 8], F16)
            nc.gpsimd.dma_start(BO16[:], bo16_d.ap()[:])

            def tree8(t1, mdst):
                """Min tree over a [128, 8192] (512 groups, 16) buffer."""
                t1v = t1[:].rearrange("p (g n) -> p g n", n=16)
                t2 = t2pool.tile([128, 4096], F16, tag="t2")
                t2v = t2[:].rearrange("p (g n) -> p g n", n=8)
                nc.vector.tensor_tensor(t2v, t1v[:, :, 0:8], t1v[:, :, 8:16], op=MIN)
                t3 = t3pool.tile([128, 2048], F16, tag="t3")
                t3v = t3[:].rearrange("p (g n) -> p g n", n=4)
                nc.vector.tensor_tensor(t3v, t2v[:, :, 0:4], t2v[:, :, 4:8], op=MIN)
                t4 = t4pool.tile([128, 1024], F16, tag="t4")
                t4v = t4[:].rearrange("p (g n) -> p g n", n=2)
                nc.vector.tensor_tensor(t4v, t3v[:, :, 0:2], t3v[:, :, 2:4], op=MIN)
                nc.vector.tensor_tensor(mdst, t4v[:, :, 0], t4v[:, :, 1], op=MIN)

            def emit_a_phase(b, half):
                XLh = iopool.tile([5, 1024], F16, tag="xl")
                nc.gpsimd.dma_start(
                    XLh[:],
                    xal_d.ap()[:, b * 2048 + half * 1024 : b * 2048 + (half + 1) * 1024],
                )
                yr = yrb[0]
                # BT[p, (i, j, c)] with c = (g1sub, n1): i-major d^T.
                BT = btpool.tile([128, 16 * 8 * 128], F16, tag="bt")
                MA = mpool.tile([128, 512], F16, tag="ma")
                t1a = t1apool.tile([128, 8192], F16, tag="t1a")
                for pair in range(4):
                    SA = sapool.tile([128, 4096], F16, tag="sa")
                    for t in range(2):
                        i = 2 * pair + t
                        lhsT = XLh[:, i * 128 : (i + 1) * 128]
                        for h in range(2):
                            D = dpool.tile([128, 1024], F32, tag="d")
                            for k in range(2):
                                nc.tensor.matmul(
                                    D[:, 512 * k : 512 * (k + 1)],
                                    lhsT,
                                    yr[:, h * 1024 + 512 * k : h * 1024 + 512 * (k + 1)],
                                    start=True,
                                    stop=True,
                                )
                            dst = SA[:, t * 2048 + 1024 * h : t * 2048 + 1024 * (h + 1)]
                            if (pair, 2 * t + h) in dve_stage:
                                nc.vector.tensor_copy(dst, D[:])
                            else:
                                nc.scalar.copy(dst, D[:])
                        # L1 fold of the two n2 halves (fp16 2x)
                        sav = SA[:, t * 2048 : (t + 1) * 2048].rearrange(
                            "p (h g n) -> p h g n", h=2, n=16
                        )
                        t1v = t1a[:].rearrange("p (t n) -> p t n", n=16)
                        nc.vector.tensor_tensor(
                            t1v[:, 64 * i : 64 * (i + 1), :],
                            sav[:, 0],
                            sav[:, 1],
                            op=MIN,
                        )
                    # one XBAR instruction: 32 blocked 128x128 transposes
                    btv = BT[:, 4096 * pair : 4096 * (pair + 1)].rearrange(
                        "p (x c) -> p x c", c=128
                    )
                    nc.sync.dma_start_transpose(btv, SA[:])
                tree8(t1a, MA[:])
                return BT, MA

            def emit_b_phase(b, half, BT, MA):
                # orientation A means: one wide matmul, BO32 stationary
                zA = zpool.tile([4, 512], F32, tag="za")
                nc.tensor.matmul(zA[:], BO32[:], MA[:], start=True, stop=True)
                zAS = zspool.tile([4, 512], F32, tag="zas")
                nc.scalar.copy(zAS[:], zA[:])
                nc.gpsimd.dma_start(za_d.ap()[b, half], zAS[:])

                # orientation B: half-batch L1 fold + min tree
                bv = BT[:].rearrange("p (g n) -> p g n", n=32)
                t1b = t1bpool.tile([128, 8192], F16, tag="t1b")
                t1bv = t1b[:].rearrange("p (g n) -> p g n", n=16)
                nc.vector.tensor_tensor(
                    t1bv, bv[:, :, 0:16], bv[:, :, 16:32], op=MIN
                )
                MB = mpool.tile([128, 512], F16, tag="mb")
                tree8(t1b, MB[:])

                # orientation B means: accumulate the n2 halves.
                # MB cols = (i, j, g1sub); j < 8 <-> (col % 64) < 32.
                zB = zpool.tile([8, 256], F32, tag="zb")
                mbv = MB[:].rearrange("p (i c) -> p i c", c=64)
                nc.tensor.matmul(
                    zB[:], BO16[:], mbv[:, :, 0:32], start=True, stop=False
                )
                nc.tensor.matmul(
                    zB[:], BO16[:], mbv[:, :, 32:64], start=False, stop=True
                )
                zBS = zspool.tile([8, 256], F32, tag="zbs")
                nc.scalar.copy(zBS[:], zB[:])
                nc.gpsimd.dma_start(zb_d.ap()[b, half], zBS[:])

            yrb = [None]
            pending = None
            for b in range(BPC):
                YRb = iopool.tile([5, 2048], F16, tag="yr")
                nc.gpsimd.dma_start(
                    YRb[:], yar_d.ap()[:, b * 2048 : (b + 1) * 2048]
                )
                yrb[0] = YRb
                for half in range(2):
                    state = emit_a_phase(b, half)
                    if pending is not None:
                        emit_b_phase(*pending)
                    pending = (b, half) + state
            emit_b_phase(*pending)

    nc.compile()
    return nc


def _host_prep(xyz1, xyz2):
    x = np.ascontiguousarray(xyz1, dtype=np.float32).reshape(B * G * N, 3)
    y = np.ascontiguousarray(xyz2, dtype=np.float32).reshape(B * G * N, 3)
    xa = np.empty((5, B * G * N), np.float16)
    xa[0] = (x * x).sum(-1)
    xa[1] = 1.0
    xa[2:5] = -2.0 * x.T
    ya = np.empty((5, B * G * N), np.float16)
    ya[0] = 1.0
    ya[1] = (y * y).sum(-1)
    ya[2:5] = y.T
    # rhs layout: (b, g, h nh) -> (b, h, g, nh)
    yar = (
        ya.reshape(5, B, G, 2, N // 2).transpose(0, 1, 3, 2, 4).reshape(5, -1)
    )
    bo32 = np.zeros((128, 4), np.float16)
    for blk in range(4):
        bo32[32 * blk : 32 * (blk + 1), blk] = 1.0 / 32
    bo16 = np.zeros((128, 8), np.float16)
    for blk in range(8):
        bo16[16 * blk : 16 * (blk + 1), blk] = 1.0 / 32
    return xa, yar, bo32, bo16


def _assemble(za, zb):
    """za [BPC,2,4,512], zb [BPC,2,8,256] -> out [BPC, 64, 64]."""
    # za[b, half, sub, (i, g2)]: g1 = half*32 + i*4 + sub
    zA = za.reshape(BPC, 2, 4, 8, 64).transpose(0, 1, 3, 2, 4).reshape(BPC, 64, 64)
    # zb[b, half, r, (i, s, sub)]: g2 = s*8 + r, g1 = half*32 + i*4 + sub
    zB = zb.reshape(BPC, 2, 8, 8, 8, 4)  # [b, half, r, i, s, sub]
    zBt = zB.transpose(0, 1, 3, 5, 4, 2).reshape(BPC, 2, 32, 64)  # [b,half,(i,sub),(s,r)]
    return zA + zBt.reshape(BPC, 64, 64)


def kernel(xyz1_matrix, xyz2_matrix):
    global LAST_EXEC_NS, LAST_RESULT
    xal, yar, bo32, bo16 = _host_prep(
        np.asarray(xyz1_matrix), np.asarray(xyz2_matrix)
    )
    nc = _build(_cfg_key(CONFIG))
    in_maps = []
    for c in range(NCORES):
        sl = slice(c * PTS, (c + 1) * PTS)
        in_maps.append(
            {
                "xal": np.ascontiguousarray(xal[:, sl]),
                "yar": np.ascontiguousarray(yar[:, sl]),
                "bo32": bo32,
                "bo16": bo16,
            }
        )
    res = bass_utils.run_bass_kernel_spmd(
        nc, in_maps, core_ids=list(range(NCORES)), trace=TRACE, tmpdir=TRACE_DIR
    )
    LAST_RESULT = res
    LAST_EXEC_NS = res.exec_time_ns
    outs = []
    for r in res.results:
        outs.append(_assemble(r["za"], r["zb"]))
    return np.concatenate(outs, axis=0).astype(np.float32)
